# revision 67
# baseline (speedup 1.0000x reference)
"""BlockKoopmanNet forward on 8 Trainium2 NeuronCores (Bass/Tile).

Data-parallel over the batch: each core handles B/8 = 2048 rows.
Everything on-device is feature-major ([feature, batch] tiles) so every
layer is a plain lhsT(=weights).T @ rhs(=activations) matmul with no
on-device transposes.

v2: the wide GEMMs (e2, d2, d3, a2, b2, b3, fpq) run as fp8e4m3
DoubleRow matmuls (two K=128 slices per instruction, 0.5 cyc/row).
Weights are pre-scaled by S=64 on the host so their ~0.02-magnitude
values land in fp8's normal range; the 1/S compensation is folded into
the consumer's activation scale (or the DT scale on the Bu path).
Activations feeding fp8 matmuls are written as fp8 directly by the
producing Silu; the z latent path (e3 head), the input layers (x, zn)
and the output head d4 stay float32r for accuracy.

Host-side preprocessing folds all the awkward structure away:
  - x / u are fed pre-transposed; x is fed twice along the partition dim
    so the K=64 input layers run as two row-packed concurrent matmuls.
  - u is fed pre-tiled 8x along features for the Bu inner product.
  - The A(x) 2x2 rotation-scale uses column-broadcast copies of the
    a_w3/e_w3 heads so exp/cos/sin and the pair shuffle become pure
    per-partition ops: cos/sin/-sin are two Sin activations with
    per-partition phase biases (pi/2 shifts), DT is folded into scales.
  - Bu = einsum('bzu,bu->bz', ...) becomes an elementwise multiply with
    the tiled u followed by a 0/1 segment-sum matmul.
  - The output is produced transposed (yT) and un-transposed on host.
"""

import sys

sys.path.insert(0, "/opt/trn_rl_repo")

import numpy as np

DT = 0.02
B, X, U, Z, H, A = 16384, 64, 16, 32, 1024, 256
N_CORES = 8
BC = B // N_CORES  # 2048 rows per core
NB = 512  # batch tile width (matmul free dim)
NCHUNK = BC // NB  # 4
SW = 64.0  # fp8 weight pre-scale

_CACHE = {}

# column offsets inside the packed f32r small-weight tensor
ROFF = {
    "d4": 0,      # 8 x 64
}
RCOLS = 512
# byte offsets inside the packed fp8 small-weight tensor
QOFF = {
    "a2": 0,      # [2, 256]
    "b2": 512,
    "fpq": 1024,  # [2, 64]
    "b3": 1152,   # [2, 512]
    "z01": 2176,  # [8, 64]
    "seg": 2688,  # [4, 32]
    "e18": 2816,  # rows 0-64 (w + bias row): [8 m, 128]
    "a18": 3840,  # rows 0-64: [2 m, 128]
    "b18": 4096,
    "d18": 4352,  # rows 0-32 (w + bias row): [8 m, 128]
}
QCOLS = 5376
BCOLS = 64


def _build(loop=None):
    import concourse.bacc as bacc
    import concourse.mybir as mybir
    from concourse.tile import TileContext
    from contextlib import nullcontext

    F32 = mybir.dt.float32
    F32R = mybir.dt.float32r
    F8 = mybir.dt.float8e4
    AF = mybir.ActivationFunctionType
    ALU = mybir.AluOpType
    DR = mybir.MatmulPerfMode.DoubleRow
    ISW = 1.0 / SW

    nc = bacc.Bacc(
        "TRN2", target_bir_lowering=False, debug=False, num_devices=N_CORES
    )

    def din(name, shape, dt=F32R):
        return nc.dram_tensor(name, shape, dt, kind="ExternalInput").ap()

    x65 = din("x65", (128, BC), F8)  # xT + ones row 64 + zero pad (bias carrier)
    uR = din("uR", (128, BC), F32)
    wpackr = din("wpackr", (128, RCOLS))
    wpack8 = din("wpack8", (128, QCOLS), F8)
    bpack = din("bpack", (128, BCOLS), F32)
    w_e2 = din("w_e2", (128, 8 * H), F8)
    w_d2 = din("w_d2", (128, 8 * H), F8)
    w_d3 = din("w_d3", (128, 8 * H), F8)
    yT = nc.dram_tensor("yT", (X, BC), F32, kind="ExternalOutput").ap()

    with TileContext(nc) as tc:
        with (
            tc.tile_pool(name="wp", bufs=1) as wp,
            tc.tile_pool(name="hp8", bufs=8) as hp8,
            tc.tile_pool(name="hpr", bufs=2) as hpr,
            tc.tile_pool(name="abp", bufs=6) as abp,
            tc.tile_pool(name="xp", bufs=3) as xp,
            tc.tile_pool(name="up", bufs=4) as up,
            tc.tile_pool(name="prp", bufs=2) as prp,
            tc.tile_pool(name="mp", bufs=2) as mp,
            tc.tile_pool(name="znp", bufs=4) as znp,
            tc.tile_pool(name="yp", bufs=2) as yp,
            tc.tile_pool(name="pbig", bufs=6, space="PSUM") as pbig,
            tc.tile_pool(name="pmid", bufs=2, space="PSUM") as pmid,
        ):
            from concourse.tile_rust import add_dep_helper

            wrt = wp.tile([128, RCOLS], F32R, tag="wrt")
            w8t = wp.tile([128, QCOLS], F8, tag="w8t")
            bpt_t = wp.tile([128, BCOLS], F32, tag="bpt")

            # input-layer weights + first x chunks FIRST so the input
            # layers start ASAP; everything else queues behind
            def xload(c):
                cs = c * NB
                ex = xp.tile([128, NB], F8, tag="x")
                nc.sync.dma_start(out=ex, in_=x65[:, cs : cs + NB])
                return ex

            early_xu = {}
            ex0 = xload(0)
            # input-layer weights (e18/a18/b18) first, rest of the pack after
            nc.sync.dma_start(
                out=w8t[:, QOFF["e18"] :], in_=wpack8[:, QOFF["e18"] :]
            )
            i_w8 = nc.sync.dma_start(
                out=w8t[:, : QOFF["e18"]], in_=wpack8[:, : QOFF["e18"]]
            )
            nc.sync.dma_start(out=bpt_t, in_=bpack)
            eu0 = up.tile([128, NB], F32, tag="u")
            nc.sync.dma_start(out=eu0, in_=uR[:, 0:NB])
            early_xu[0] = (ex0, eu0)
            ex1 = xload(1)
            eu1 = up.tile([128, NB], F32, tag="u")
            nc.sync.dma_start(out=eu1, in_=uR[:, NB : 2 * NB])
            early_xu[1] = (ex1, eu1)
            i_wp = nc.sync.dma_start(out=wrt, in_=wpackr)
            bpt = bpt_t[:]

            # big weight matrices: idle gpsimd queue, held behind the small
            # pack (a gated DMA parks its whole issuing queue, so they must
            # not share a queue with compute-critical work)
            def wload8(ap, tag, dep):
                t = wp.tile([128, 8, H], F8, tag=tag)
                apv = ap.rearrange("p (k m) -> p k m", k=8)
                ia = nc.gpsimd.dma_start(out=t[:, :, : H // 2], in_=apv[:, :, : H // 2])
                add_dep_helper(ia.ins, dep.ins, reason="after small weights")
                ib = nc.gpsimd.dma_start(out=t[:, :, H // 2 :], in_=apv[:, :, H // 2 :])
                add_dep_helper(ib.ins, dep.ins, reason="after small weights")
                return t

            e2w = wload8(w_e2, "e2w", i_wp)
            d2w = wload8(w_d2, "d2w", i_wp)
            d3w = wload8(w_d3, "d3w", i_wp)

            wv = wrt[:]

            class PackedWR:
                """f32r packed heads: [128, kc, M] views into wrt."""

                def __init__(self, name, M):
                    self.name, self.M = name, M

                def __getitem__(self, idx):
                    _, k, ms = idx
                    o = ROFF[self.name] + k * self.M
                    lo = ms.start or 0
                    hi = self.M if ms.stop is None else ms.stop
                    return wv[:, o + lo : o + hi]

            d4w = PackedWR("d4", X)

            w8v = w8t[:]
            a2w = w8v[:, QOFF["a2"] : QOFF["a2"] + 512].rearrange(
                "p (k m) -> p k m", k=2
            )
            b2w = w8v[:, QOFF["b2"] : QOFF["b2"] + 512].rearrange(
                "p (k m) -> p k m", k=2
            )
            fpqw = w8v[:, QOFF["fpq"] : QOFF["fpq"] + 128].rearrange(
                "p (k m) -> p k m", k=2
            )
            b3w = w8v[:, QOFF["b3"] : QOFF["b3"] + 1024].rearrange(
                "p (k m) -> p k m", k=2
            )
            z01w = w8v[:, QOFF["z01"] : QOFF["z01"] + 512].rearrange(
                "p (k m) -> p k m", k=8
            )
            seg8 = w8v[:, QOFF["seg"] : QOFF["seg"] + 128].rearrange(
                "p (k m) -> p k m", k=4
            )
            e18w = w8v[:, QOFF["e18"] : QOFF["e18"] + 1024].rearrange(
                "p (m c) -> p m c", m=8
            )
            a18w = w8v[:, QOFF["a18"] : QOFF["a18"] + 256].rearrange(
                "p (m c) -> p m c", m=2
            )
            b18w = w8v[:, QOFF["b18"] : QOFF["b18"] + 256].rearrange(
                "p (m c) -> p m c", m=2
            )
            d18w = w8v[:, QOFF["d18"] : QOFF["d18"] + 1024].rearrange(
                "p (m c) -> p m c", m=8
            )

            e1b = bpt[:, 0:8]
            e2b = bpt[:, 8:16]
            a1b = bpt[:, 16:18]
            a2b = bpt[:, 18:20]
            b1b = bpt[:, 20:22]
            b2b = bpt[:, 22:24]
            b3b = bpt[:, 24:28]   # pre-scaled by SW on host
            d1b = bpt[:, 28:36]
            d2b = bpt[:, 36:44]
            d3b = bpt[:, 44:52]
            z01b = bpt[:64, 52:53]
            kA = bpt[:32, 53:54]
            kB = bpt[:32, 54:55]
            sAe = bpt[:32, 55:56]
            sAo = bpt[:32, 56:57]
            msAe = bpt[:32, 57:58]
            d4b = bpt[:64, 58:59]

            def mlp_layer_dr(w_t, kp_n, b_t, rhs_t, h_out, mi_lo, mi_hi, scale=ISW):
                """h_out[:, mi, :] = silu(scale * sum_kp DR(w, rhs) + b)."""
                for mi in range(mi_lo, mi_hi):
                    ps = pbig.tile([128, NB], F32, tag="pb")
                    for kp in range(kp_n):
                        nc.tensor.matmul(
                            ps,
                            w_t[:, 2 * kp : 2 * kp + 2, mi * 128 : (mi + 1) * 128],
                            rhs_t[:, 2 * kp : 2 * kp + 2, :],
                            start=(kp == 0),
                            stop=(kp == kp_n - 1),
                            perf_mode=DR,
                        )
                    nc.scalar.activation(
                        h_out[:, mi, :], ps, AF.Silu,
                        bias=b_t[:, mi : mi + 1], scale=scale,
                    )

            def in_layer(w3, m_n, x_t, h_out):
                """Input layer from x: plain fp8 K=65 matmul per m-chunk
                (FWL-eligible); bias rides the ones-row of x."""
                for mi in range(m_n):
                    ps = pbig.tile([128, NB], F32, tag="pb")
                    nc.tensor.matmul(
                        ps, w3[:, mi, :], x_t[:],
                        start=True, stop=True,
                    )
                    nc.scalar.activation(
                        h_out[:, mi, :], ps, AF.Silu, bias=0.0, scale=ISW,
                    )

            loop_ctx = tc.For_i(0, loop, 1) if loop is not None else nullcontext()
            with loop_ctx:
                zn_tiles = []
                p1out = {}

                # pass 1 (input layers, x-only): software-pipelined one chunk
                # ahead of pass 2 so the pass-2 heads never wait on the ACT
                # backlog of the previous chunk's Silus
                def pass1(c):
                    cs = c * NB
                    if loop is None and c in early_xu:
                        x_t, u_t = early_xu[c]
                    else:
                        # late chunks load via the (idle) gpsimd queue to
                        # keep the sync sequencer off the critical path
                        x_t = xp.tile([128, NB], F8, tag="x")
                        nc.gpsimd.dma_start(out=x_t, in_=x65[:, cs : cs + NB])
                        u_t = up.tile([128, NB], F32, tag="u")
                        nc.gpsimd.dma_start(out=u_t, in_=uR[:, cs : cs + NB])

                    h1 = hp8.tile([128, 8, NB], F8, tag="h8")
                    in_layer(e18w, 8, x_t, h1)
                    ha1 = abp.tile([128, 2, NB], F8, tag="ab")
                    in_layer(a18w, 2, x_t, ha1)
                    hb1 = abp.tile([128, 2, NB], F8, tag="ab")
                    in_layer(b18w, 2, x_t, hb1)
                    p1out[c] = (u_t, h1, ha1, hb1)

                # pass 2: encoder body + heads + latent step
                def pass2(c):
                    u_t, h1, ha1, hb1 = p1out.pop(c)

                    ha2 = abp.tile([128, 2, NB], F8, tag="ab")
                    mlp_layer_dr(a2w, 1, a2b, ha1, ha2, 0, 2)
                    hb2 = abp.tile([128, 2, NB], F8, tag="ab")
                    mlp_layer_dr(b2w, 1, b2b, hb1, hb2, 0, 2)

                    # big encoder layer, with the F|P|Q head block slotted
                    # in after two m-chunks
                    h2 = hp8.tile([128, 8, NB], F8, tag="h8")
                    mlp_layer_dr(e2w, 4, e2b, h1, h2, 0, 2)

                    # A(x) head: rows 0-31 of pfpq = a per pair (dup), rows
                    # 32-63 = b.  |a*DT|,|b*DT| <~ 1e-3, so exp/cos/sin
                    # linearize (err ~1e-6): zn0 = (1+DT+DT^2 a) z0 - DT^2 b z1
                    # + DT Bu0, zn1 mirrored.  Row-interleaved coefficients
                    # A,B built with per-partition masks -- pure DVE, no ACT.
                    pfpq = pmid.tile([2 * Z, NB], F32, tag="pm")
                    nc.tensor.matmul(
                        pfpq, fpqw[:, 0:2, :], ha2[:, 0:2, :],
                        start=True, stop=True, perf_mode=DR,
                    )
                    A_t = mp.tile([Z, NB], F32, tag="A")
                    nc.vector.tensor_scalar(
                        out=A_t[:], in0=pfpq[:Z], scalar1=sAe, scalar2=kA,
                        op0=ALU.mult, op1=ALU.add,
                    )
                    nc.vector.scalar_tensor_tensor(
                        out=A_t[:], in0=pfpq[Z:], scalar=sAo, in1=A_t[:],
                        op0=ALU.mult, op1=ALU.add,
                    )
                    B_t = mp.tile([Z, NB], F32, tag="B")
                    nc.vector.tensor_scalar(
                        out=B_t[:], in0=pfpq[:Z], scalar1=sAo, scalar2=kB,
                        op0=ALU.mult, op1=ALU.add,
                    )
                    nc.vector.scalar_tensor_tensor(
                        out=B_t[:], in0=pfpq[Z:], scalar=msAe, in1=B_t[:],
                        op0=ALU.mult, op1=ALU.add,
                    )

                    mlp_layer_dr(e2w, 4, e2b, h1, h2, 2, 8)

                    # z pair-broadcasts Z0|Z1 in one [64, NB] psum (fp8 DR)
                    pz = pmid.tile([2 * Z, NB], F32, tag="pm")
                    for kp in range(4):
                        nc.tensor.matmul(
                            pz, z01w[:, 2 * kp : 2 * kp + 2, :],
                            h2[:, 2 * kp : 2 * kp + 2, :],
                            start=(kp == 0), stop=(kp == 3), perf_mode=DR,
                        )
                    z0_t = mp.tile([Z, NB], F32, tag="Z0")
                    nc.vector.tensor_scalar(
                        out=z0_t[:], in0=pz[:Z], scalar1=1.0 / SW,
                        scalar2=z01b[:Z, 0:1], op0=ALU.mult, op1=ALU.add,
                    )
                    z1_t = mp.tile([Z, NB], F32, tag="Z1")
                    nc.vector.tensor_scalar(
                        out=z1_t[:], in0=pz[Z:], scalar1=1.0 / SW,
                        scalar2=z01b[Z:, 0:1], op0=ALU.mult, op1=ALU.add,
                    )

                    # Bflat + Bu (psb = SW * pre-bias; b3b pre-scaled by SW);
                    # prods written as fp8 pairs for the DR segment-sum
                    pr2 = prp.tile([128, 4, NB], F8, tag="prod")
                    for mc in range(4):
                        psb = pbig.tile([128, NB], F32, tag="pb")
                        nc.tensor.matmul(
                            psb, b3w[:, 0:2, mc * 128 : (mc + 1) * 128],
                            hb2[:, 0:2, :],
                            start=True, stop=True, perf_mode=DR,
                        )
                        nc.vector.scalar_tensor_tensor(
                            out=pr2[:, mc, :], in0=psb[:],
                            scalar=b3b[:, mc : mc + 1],
                            in1=u_t[:], op0=ALU.add, op1=ALU.mult,
                        )
                    pbu = pmid.tile([Z, NB], F32, tag="pm")
                    for hh in range(2):
                        nc.tensor.matmul(
                            pbu, seg8[:, 2 * hh : 2 * hh + 2, :],
                            pr2[:, 2 * hh : 2 * hh + 2, :],
                            start=(hh == 0), stop=(hh == 1), perf_mode=DR,
                        )

                    # z_next = A*Z0 + B*Z1 + (DT/SW)*pbu   (in-place DVE)
                    nc.vector.tensor_tensor(
                        out=A_t[:], in0=A_t[:], in1=z0_t[:], op=ALU.mult
                    )
                    nc.vector.tensor_tensor(
                        out=B_t[:], in0=B_t[:], in1=z1_t[:], op=ALU.mult
                    )
                    nc.vector.tensor_tensor(
                        out=A_t[:], in0=A_t[:], in1=B_t[:], op=ALU.add
                    )
                    zn_t = znp.tile([128, NB], F8, tag="zn")
                    for pz0 in (32, 64, 96):  # zero pad for K=128
                        nc.gpsimd.memset(zn_t[pz0 : pz0 + 32, :], 0.0)
                    nc.gpsimd.memset(zn_t[32:33, :], 1.0)  # d1 bias carrier
                    nc.vector.scalar_tensor_tensor(
                        out=zn_t[:Z], in0=pbu[:], scalar=DT / SW, in1=A_t[:],
                        op0=ALU.mult, op1=ALU.add,
                    )
                    zn_tiles.append(zn_t)

                # phase B chunk: decoder stack (ACT-light, PE-heavy)
                def passB(c):
                    cs = c * NB
                    zn_t = zn_tiles[c]
                    hd1 = hp8.tile([128, 8, NB], F8, tag="h8")
                    in_layer(d18w, 8, zn_t, hd1)
                    hd2 = hp8.tile([128, 8, NB], F8, tag="h8")
                    mlp_layer_dr(d2w, 4, d2b, hd1, hd2, 0, 8)
                    hd3 = hpr.tile([128, 8, NB], F32R, tag="hr")
                    mlp_layer_dr(d3w, 4, d3b, hd2, hd3, 0, 8)

                    py_full = pbig.tile([128, NB], F32, tag="pb")
                    py_t = py_full[:X]
                    for k in range(8):
                        nc.tensor.matmul(
                            py_t, d4w[:, k, :], hd3[:, k, :],
                            start=(k == 0), stop=(k == 7),
                        )
                    y_sb = yp.tile([X, NB], F32, tag="y")
                    nc.vector.tensor_scalar_add(
                        out=y_sb[:], in0=py_t[:], scalar1=d4b
                    )
                    nc.gpsimd.dma_start(out=yT[:, cs : cs + NB], in_=y_sb)

                # schedule: pass 1 one chunk ahead of pass 2; decoder chunks
                # interleaved so the ACT-saturated encoder blocks alternate
                # with ACT-light decoder blocks
                pass1(0)
                pass1(1)
                pass2(0)
                pass1(2)
                pass2(1)
                passB(0)
                pass1(3)
                pass2(2)
                passB(1)
                pass2(3)
                passB(2)
                passB(3)

    nc.compile()
    return nc


def _prep_host(inputs):
    import ml_dtypes

    f32 = np.float32
    FP8 = ml_dtypes.float8_e4m3
    x = np.asarray(inputs["x"], f32)
    u = np.asarray(inputs["u"], f32)

    xT = np.ascontiguousarray(x.T)
    # [65, B] fp8: xT + ones row 64 so layer biases ride the matmul
    x65 = np.zeros((128, B), FP8)
    x65[0:64] = xT.astype(FP8)
    x65[64] = np.float32(1.0)
    uR = np.tile(np.ascontiguousarray(u.T), (8, 1))  # [128, B]

    def in_pack(w, b):
        """[K, M] + bias -> [128, M] fp8 (SW-scaled, bias row K, zero pad)."""
        w = np.asarray(w, f32)
        k = w.shape[0]
        out = np.zeros((128, w.shape[1]), FP8)
        out[0:k] = (w * SW).astype(FP8)
        out[k] = (np.asarray(b, f32) * SW).astype(FP8)
        return out

    def fm(w):
        """[K, M] -> [128, (K//128)*M]: per-partition-contiguous lhsT chunks."""
        kc = w.shape[0] // 128
        return np.ascontiguousarray(
            w.reshape(kc, 128, w.shape[1]).transpose(1, 0, 2).reshape(128, -1)
        )

    def fm8(w):
        """fm() of SW-scaled fp8 quantized weights."""
        q = (np.asarray(w, f32) * SW).astype(FP8)
        kc = q.shape[0] // 128
        return np.ascontiguousarray(
            q.reshape(kc, 128, q.shape[1]).transpose(1, 0, 2).reshape(128, -1)
        )

    idx0 = np.arange(Z) // 2 * 2
    idx1 = idx0 + 1
    even = (np.arange(Z) % 2 == 0).astype(f32)

    e_w3 = np.asarray(inputs["e_w3"], f32)
    e_b3 = np.asarray(inputs["e_b3"], f32)
    a_w3 = np.asarray(inputs["a_w3"], f32)
    a_b3 = np.asarray(inputs["a_b3"], f32)

    seg8 = np.zeros((128, 4, 32), f32)
    for mc in range(4):
        for k in range(128):
            seg8[k, mc, 8 * mc + k // 16] = 1.0

    pi = np.pi

    wpackr = np.zeros((128, RCOLS), f32)
    wpackr[:, ROFF["d4"] : ROFF["d4"] + 512] = fm(np.asarray(inputs["d_w4"], f32))

    wpack8 = np.zeros((128, QCOLS), FP8)
    wpack8[:, QOFF["a2"] : QOFF["a2"] + 512] = fm8(inputs["a_w2"])
    wpack8[:, QOFF["b2"] : QOFF["b2"] + 512] = fm8(inputs["b_w2"])
    wpack8[:, QOFF["fpq"] : QOFF["fpq"] + 128] = fm8(
        np.concatenate([a_w3[:, idx0], a_w3[:, idx1]], axis=1)
    )
    wpack8[:, QOFF["b3"] : QOFF["b3"] + 1024] = fm8(inputs["b_w3"])
    wpack8[:, QOFF["z01"] : QOFF["z01"] + 512] = fm8(
        np.concatenate([e_w3[:, idx0], e_w3[:, idx1]], axis=1)
    )
    wpack8[:, QOFF["seg"] : QOFF["seg"] + 128] = seg8.reshape(128, 128).astype(FP8)
    wpack8[:, QOFF["e18"] : QOFF["e18"] + 1024] = in_pack(
        inputs["e_w1"], inputs["e_b1"]
    )
    wpack8[:, QOFF["a18"] : QOFF["a18"] + 256] = in_pack(
        inputs["a_w1"], inputs["a_b1"]
    )
    wpack8[:, QOFF["b18"] : QOFF["b18"] + 256] = in_pack(
        inputs["b_w1"], inputs["b_b1"]
    )
    wpack8[:, QOFF["d18"] : QOFF["d18"] + 1024] = in_pack(
        inputs["d_w1"], inputs["d_b1"]
    )

    def bcol(b):
        return np.asarray(b, f32).reshape(-1, 128).T

    bpack = np.zeros((128, BCOLS), f32)
    bpack[:, 0:8] = bcol(inputs["e_b1"])
    bpack[:, 8:16] = bcol(inputs["e_b2"])
    bpack[:, 16:18] = bcol(inputs["a_b1"])
    bpack[:, 18:20] = bcol(inputs["a_b2"])
    bpack[:, 20:22] = bcol(inputs["b_b1"])
    bpack[:, 22:24] = bcol(inputs["b_b2"])
    bpack[:, 24:28] = bcol(inputs["b_b3"]) * SW
    bpack[:, 28:36] = bcol(inputs["d_b1"])
    bpack[:, 36:44] = bcol(inputs["d_b2"])
    bpack[:, 44:52] = bcol(inputs["d_b3"])
    # linearized A(x) coefficients: zn0 = (1+DT+DT^2 a) z0 - DT^2 b z1 + ...
    odd = 1.0 - even
    ab3_0 = a_b3[idx0]
    ab3_1 = a_b3[idx1]
    bpack[:64, 52] = np.concatenate([e_b3[idx0], e_b3[idx1]])
    bpack[:32, 53] = even * (1 + DT) + DT * DT * (even * ab3_0 + odd * ab3_1)
    bpack[:32, 54] = odd * (1 + DT) + DT * DT * (odd * ab3_0 - even * ab3_1)
    bpack[:32, 55] = even * (DT * DT / SW)
    bpack[:32, 56] = odd * (DT * DT / SW)
    bpack[:32, 57] = -even * (DT * DT / SW)
    bpack[:64, 58] = np.asarray(inputs["d_b4"], f32)

    shared = {
        "wpackr": wpackr,
        "wpack8": wpack8,
        "bpack": bpack,
        "w_e2": fm8(inputs["e_w2"]),
        "w_d2": fm8(inputs["d_w2"]),
        "w_d3": fm8(inputs["d_w3"]),
    }

    in_maps = []
    for c in range(N_CORES):
        sl = slice(c * BC, (c + 1) * BC)
        m = dict(shared)
        m["x65"] = np.ascontiguousarray(x65[:, sl])
        m["uR"] = np.ascontiguousarray(uR[:, sl])
        in_maps.append(m)
    return in_maps


def kernel(**inputs) -> np.ndarray:
    from concourse import bass_utils

    if "nc" not in _CACHE:
        _CACHE["nc"] = _build()
    nc = _CACHE["nc"]
    in_maps = _prep_host(inputs)
    res = bass_utils.run_bass_kernel_spmd(
        nc, in_maps, core_ids=list(range(N_CORES))
    )
    return np.concatenate(
        [np.asarray(res.results[c]["yT"]).T for c in range(N_CORES)], axis=0
    ).astype(np.float32)
